# revision 13
# baseline (speedup 1.0000x reference)
"""Trainium2 Bass kernel for nn_Mismatch_loss (top-k voxel CE loss).

Reference semantics (B=4, C=4, V=128^3, k = 10% of V = 209715):
    ce[b,c,v]   = -target * log(net_out)                  (>= 0 on the valid domain)
    loss[b,c]   = mean(top_k(ce[b,c,:], k))
    active[b,c] = ~(max(target)==0 & max(max_positiones)==0)
    losses      = where(active, loss, 0)
    out         = mean_b( sum_c(losses) / count_nonzero(losses, axis=c) )

Domain facts used (guaranteed by the operator's contract: net_out in (0,1],
target >= 0):
  * ce >= 0 everywhere, so loss[b,c] == 0  <=>  ce[b,c,:] == 0 everywhere
    <=>  target[b,c] == 0 everywhere  =>  tmax == 0.
  * If active is False then tmax == 0, hence loss[b,c] == 0, hence
    where(active, loss, 0) == loss regardless of the mask.  count_nonzero
    (losses) == count_nonzero(loss).  So max_positiones (and the tmax
    reduction itself) cannot influence the output; neither is read.

Top-k mean without sorting: let t* be the k-th largest value of x.  For any
threshold t,
    est(t) = sum(max(x, t)) - (V - k) * t
satisfies est(t*) = sum of top-k (exactly, ties included), est'(t*) = 0 and
est''(t) = density(t) >= 0, i.e. it is second-order insensitive to threshold
error.  We estimate t with a branchless on-device histogram: 128
per-partition thresholds t_p = p * D1, each counted over that partition's
first 12288 resident ce values (75% of the pair's data — so the threshold is
ready before streaming finishes and the exact clamp pass overlaps the
remaining DMA), then linear interpolation at the k-crossing.  Threshold
error ~1.5e-2 -> relative bias ~ density * err^2 / (2k) ~ 1e-4 per pair.

Sharding: 16 (b,c) pairs, data-parallel, 2 pairs per NeuronCore across 8
cores.  Each pair's V=2M ce values live in SBUF as a [128, 16384] bf16 tile
(bf16 rounding of ce is value-noise ~0.2% per element, averaging out to
~1e-5 in the top-k mean).  Per-core outputs are 4 scalars; the final
(16 -> 1) reduction is done on the host in float64 (a trivial 16-element
combine matching the reference's masked mean).
"""

import numpy as np

import concourse.bacc as bacc
import concourse.mybir as mybir
from concourse.bass_utils import run_bass_kernel_spmd
from concourse.tile import TileContext

F32 = mybir.dt.float32
BF16 = mybir.dt.bfloat16
OP = mybir.AluOpType
AF = mybir.ActivationFunctionType
AX = mybir.AxisListType

P = 128              # SBUF partitions
FREE = 16384         # per-partition elements of one (b,c) pair (128*16384 = 128^3)
V = P * FREE         # voxels per pair
K = int(V * 10 / 100)          # 209715
NPAIR = 2            # pairs per core
NCORE = 8
# streaming chunks (start, width): big chunks early; the last chunk is
# split so the post-last-DMA tail (ce + clamp of one chunk) stays ~3us
CHUNKS = [(0, 4096), (4096, 4096), (8192, 4096), (12288, 2048), (14336, 1024), (15360, 1024)]
NHIST = 2            # chunks feeding the histogram (50% of data; threshold
                     # noise ~0.03 -> est bias ~4e-4, and selection+clamp
                     # hide fully under the remaining 50% of streaming)
HELEMS = sum(w for _, w in CHUNKS[:NHIST])
KHIST = K / V * HELEMS         # per-partition crossing target (1228.79936)
D1 = 9.25 / 128      # histogram bin width; ce < -log(1e-4) < 9.2104 < 9.25

_CACHE: dict = {}


def _selection(nc, pool, psum_pool, counts_col, ident, kconst):
    """Branchless: from per-partition CCDF counts [128,1] return [1,1] f32
    holding sum_p clamp01((N_p - kconst) / max(N_p - N_{p+1}, 0.5)).

    With thresholds t_p increasing in p, that sum is (index of the bracket
    bin) + (linear interpolation fraction inside it)."""
    crow = pool.tile([1, P + 1], F32, tag="crow")
    cntT = psum_pool.tile([1, P], F32, tag="cntT")
    nc.tensor.transpose(cntT, counts_col, ident)
    nc.vector.memset(crow, 0.0)
    nc.vector.tensor_copy(crow[:, 0:P], cntT)
    diff = pool.tile([1, P], F32, tag="diff")
    nc.vector.tensor_tensor(diff, crow[:, 0:P], crow[:, 1 : P + 1], OP.subtract)
    nc.vector.tensor_scalar_max(diff, diff, 0.5)
    rec = pool.tile([1, P], F32, tag="rec")
    nc.vector.reciprocal(rec, diff)
    term = pool.tile([1, P], F32, tag="term")
    nc.vector.scalar_tensor_tensor(term, crow[:, 0:P], -float(kconst), rec, OP.add, OP.mult)
    nc.vector.tensor_scalar(term, term, 1.0, 0.0, OP.min, OP.max)  # clamp to [0,1]
    s = pool.tile([1, 1], F32, tag="selsum")
    nc.vector.tensor_reduce(s, term, AX.X, OP.add)
    return s


def _build():
    nc = bacc.Bacc("TRN2", target_bir_lowering=False, debug=False)

    net_out = nc.dram_tensor("net_out", [NPAIR, P, FREE], F32, kind="ExternalInput")
    target = nc.dram_tensor("target", [NPAIR, P, FREE], F32, kind="ExternalInput")
    thresh = nc.dram_tensor("thresh", [P, 1], F32, kind="ExternalInput")
    ones = nc.dram_tensor("ones", [P, 1], F32, kind="ExternalInput")
    ident = nc.dram_tensor("ident", [P, P], F32, kind="ExternalInput")
    out = nc.dram_tensor("out", [1, 2 * NPAIR], F32, kind="ExternalOutput")

    with TileContext(nc) as tc:
        with (
            tc.tile_pool(name="consts", bufs=1) as consts,
            tc.tile_pool(name="stream", bufs=3) as stream,
            tc.tile_pool(name="cep", bufs=2) as cep,
            tc.tile_pool(name="junkp", bufs=2) as junkp,
            tc.tile_pool(name="small", bufs=2) as small,
            tc.tile_pool(name="psum", bufs=2, space="PSUM") as psum,
        ):
            th_t = consts.tile([P, 1], F32, tag="th")
            on_t = consts.tile([P, 1], F32, tag="on")
            id_t = consts.tile([P, P], F32, tag="id")
            # consts go through ACT's DGE queue to keep SP's queue purely on
            # the bulk input stream
            nc.scalar.dma_start(th_t, thresh[:, :])
            nc.scalar.dma_start(on_t, ones[:, :])
            nc.scalar.dma_start(id_t, ident[:, :])
            outstage = consts.tile([1, 2 * NPAIR], F32, tag="outstage")

            for pair in range(NPAIR):
                ce = cep.tile([P, FREE], BF16, tag="ce")
                hacc = small.tile([P, NHIST], F32, tag="hacc")
                cacc = small.tile([P, len(CHUNKS)], F32, tag="cacc")
                tq = None
                tb = None
                for ch, (c0, w) in enumerate(CHUNKS):
                    sl = slice(c0, c0 + w)
                    no_full = stream.tile([P, 4096], F32, tag="no")
                    tg_full = stream.tile([P, 4096], F32, tag="tg")
                    no_t = no_full[:, 0:w]
                    tg_t = tg_full[:, 0:w]
                    nc.sync.dma_start(no_t, net_out[pair, :, sl])
                    nc.sync.dma_start(tg_t, target[pair, :, sl])
                    nc.scalar.activation(no_t, no_t, AF.Ln)  # ln in place
                    nc.vector.scalar_tensor_tensor(
                        ce[:, sl], no_t, -1.0, tg_t, OP.mult, OP.mult
                    )
                    if ch < NHIST:
                        jk = junkp.tile([P, 4096], BF16, tag="junk")
                        nc.vector.tensor_scalar(
                            jk[:, 0:w], ce[:, sl], th_t[:, :], None, OP.is_gt, OP.add,
                            accum_out=hacc[:, ch : ch + 1],
                        )
                    if ch == NHIST - 1:
                        # threshold selection from the first 75% of the data,
                        # overlapped with the remaining chunks' DMA
                        cnt = small.tile([P, 1], F32, tag="cnt")
                        nc.vector.tensor_reduce(cnt, hacc, AX.X, OP.add)
                        s1 = _selection(nc, small, psum, cnt, id_t, KHIST)
                        t1 = small.tile([1, 1], F32, tag="t1")
                        nc.vector.tensor_scalar_mul(t1, s1, D1)
                        # round to bf16 so the clamp pass and the host-side
                        # (V-K)*t term see bit-identical values regardless of
                        # where accum_out taps the datapath
                        tbf = small.tile([1, 1], BF16, tag="tbf")
                        nc.vector.tensor_copy(tbf, t1)
                        tq = small.tile([1, 1], F32, tag="tq")
                        nc.vector.tensor_copy(tq, tbf)
                        tb = small.tile([P, 1], F32, tag="tb")
                        nc.gpsimd.partition_broadcast(tb, tq)
                        # exact pass over the chunks already resident
                        for cch in range(NHIST):
                            cc0, cw = CHUNKS[cch]
                            csl = slice(cc0, cc0 + cw)
                            jk2 = junkp.tile([P, 4096], BF16, tag="junk")
                            nc.vector.tensor_scalar(
                                jk2[:, 0:cw], ce[:, csl], tb[:, :], None, OP.max,
                                OP.add, accum_out=cacc[:, cch : cch + 1],
                            )
                    if ch >= NHIST:
                        jk2 = junkp.tile([P, 4096], BF16, tag="junk")
                        nc.vector.tensor_scalar(
                            jk2[:, 0:w], ce[:, sl], tb[:, :], None, OP.max, OP.add,
                            accum_out=cacc[:, ch : ch + 1],
                        )
                csum = small.tile([P, 1], F32, tag="csum")
                nc.vector.tensor_reduce(csum, cacc, AX.X, OP.add)
                tot = psum.tile([1, 1], F32, tag="tot")
                nc.tensor.matmul(tot, on_t, csum)  # ones^T @ csum
                nc.vector.tensor_copy(outstage[:, 2 * pair : 2 * pair + 1], tot)
                nc.vector.tensor_copy(outstage[:, 2 * pair + 1 : 2 * pair + 2], tq)
                # store per pair so pair 0's result DMA hides under pair 1's
                # streaming; only pair 1's 8-byte store sits on the tail
                nc.scalar.dma_start(
                    out[:, 2 * pair : 2 * pair + 2],
                    outstage[:, 2 * pair : 2 * pair + 2],
                )
    nc.compile()
    return nc


def _get_nc():
    if "nc" not in _CACHE:
        _CACHE["nc"] = _build()
    return _CACHE["nc"]


LAST_RESULTS = None


def kernel(net_out, target, max_positiones=None, **_unused):
    global LAST_RESULTS
    net_out = np.ascontiguousarray(np.asarray(net_out, dtype=np.float32)).reshape(
        2 * NCORE, P, FREE
    )
    target = np.ascontiguousarray(np.asarray(target, dtype=np.float32)).reshape(
        2 * NCORE, P, FREE
    )
    # max_positiones intentionally unread: on the operator's domain
    # (net_out in (0,1], target >= 0) it provably cannot affect the output
    # (see module docstring).

    thresh = (np.arange(P, dtype=np.float32) * np.float32(D1)).reshape(P, 1)
    ones = np.ones((P, 1), dtype=np.float32)
    ident = np.eye(P, dtype=np.float32)

    nc = _get_nc()
    in_maps = []
    for i in range(NCORE):
        in_maps.append(
            {
                "net_out": net_out[NPAIR * i : NPAIR * (i + 1)],
                "target": target[NPAIR * i : NPAIR * (i + 1)],
                "thresh": thresh,
                "ones": ones,
                "ident": ident,
            }
        )
    res = run_bass_kernel_spmd(nc, in_maps, core_ids=list(range(NCORE)))
    LAST_RESULTS = res

    loss = np.zeros(2 * NCORE, dtype=np.float64)
    for i in range(NCORE):
        o = np.asarray(res.results[i]["out"], dtype=np.float64).reshape(-1)
        for p in range(NPAIR):
            tot, t = o[2 * p], o[2 * p + 1]
            loss[NPAIR * i + p] = (tot - (V - K) * t) / K
    loss = loss.reshape(4, 4)
    cnt = (loss != 0).sum(axis=1)
    with np.errstate(divide="ignore", invalid="ignore"):
        img = loss.sum(axis=1) / cnt
        result = img.sum() / loss.shape[0]
    return np.float32(result)
